# revision 32
# baseline (speedup 1.0000x reference)
"""Trainium2 Bass kernel for nn_DGraFormer_framework (gnn_message_passing).

Reference computation (B=32, N=64, S=336, D=32, K=3 layers, beta=0.05):
    per (b, s):  A = adj[b,s]  (row-normalized [N,N])
    H0 = x w_start + b_start          [N, D]
    H_{k+1} = beta*x + (1-beta) A^T H_k
    out = concat(H_0..H_3) @ w_mlp + b_mlp   -> [b, n, s]

Everything is linear in the feature dim, so D collapses:
    out[b,:,s] = pre0 + A'(pre1 + A'(pre2 + A' pre3))      (Horner)
where A' = A^T and pre_j[b,n,s] = c_j * x[b,n,s] + d_j (scalars c_j, d_j, e
derived from w_start/b_start/w_mlp/b_mlp on the host; e folded into pre0).

Quantization: adj is stored fp8 (e3m4) scaled by 16 to land in the normal
range. The scale is folded out for free: pre_j is pre-scaled by 16^(3-j)
on the host (so each pass's psum and the V tiles carry a growing power of
16) and the final output is divided by 16^3 after the gather. All scaled
magnitudes stay well inside fp16 range.

Device kernel (per core; data-parallel over batch, 4 b per core):
  - adj[b] (336 s-slices of [64,64]) packed as 84 "quads": 4 A-matrices per
    128x128 stationary tile (2x2 blocks of 64x64), fp8 e3m4.
  - 3 passes; pass k: one matmul per quad, moving operand [128,4] whose col
    4q+j carries chain s=4q+j's vector in one 64-partition half (zeros in the
    other).  Quad block (pb,cb) holds A_{s=4q+sigma(pb,cb)},
    sigma = [[1,0],[2,3]][pb][cb] (cb=0: s=4q+1+pb; cb=1: s=4q+3*pb), so
    col j classes: j0:(0,1) in-top/out-bot, j1:(0,0) top/top,
    j2:(1,0) in-bot/out-top, j3:(1,1) bot/bot.
  - Transitions between passes are batched strided DVE adds (psum + pre -> V);
    the two "crossed" classes (j0, j2) bounce through a shift-matmul whose
    stationary is the 64<->64 partition block swap matrix.
  - Final pass adds pre0 and lands all four classes in a contiguous [64,336]
    fp16 output tile (col order == s order), one DMA per b back to HBM.
  - All input DMAs are issued up front (adj for all 4 batches lives in SBUF
    simultaneously; fp8 makes that 5.5 MB). Batch 0's adj is chunked so the
    first matmuls can start after ~130 KB has landed. The V3 start vectors
    are staged host-side (zeros in the complementary halves) and DMA'd
    directly, so no on-device init pass is needed.
"""

import sys

sys.path.insert(0, "/opt/trn_rl_repo")

import ml_dtypes
import numpy as np

import concourse.bass as bass
import concourse.mybir as mybir
import concourse.tile as tile
from concourse import bacc
from concourse.bass_utils import run_bass_kernel_spmd

B, N, S, D = 32, 64, 336, 32
MP_LAYERS = 3
PROPBETA = 0.05
NCORES = 8
BL = B // NCORES          # batches per core
Q = S // 4                # quads per batch (84)

ADJ_DT = mybir.dt.float8e3    # e3m4
ADJ_NP = ml_dtypes.float8_e3m4
ADJ_SCALE = 16.0
OUT_DESCALE = float(ADJ_SCALE ** MP_LAYERS)
V_DT = mybir.dt.float16       # chain-vector / pre / out dtype
V_NP = np.float16

f32 = mybir.dt.float32


def _coefficients(w_start, b_start, w_mlp, b_mlp):
    """Collapse the feature dim: out = sum_j A'^j (c_j x + d_j 1) + e (j=0..K).

    H_k = sum_j A'^j (x u_{k,j}^T + 1 v_{k,j}^T) with
    H_0: u=w_start, v=b_start;  H_{k+1} = beta x 1^T + (1-beta) A' H_k.
    """
    K = MP_LAYERS
    beta, sb = PROPBETA, 1.0 - PROPBETA
    ws = w_start[0].astype(np.float64)
    bs = b_start.astype(np.float64)
    w = [w_mlp[k * D:(k + 1) * D, 0].astype(np.float64) for k in range(K + 1)]

    u = {(0, 0): ws}
    v = {(0, 0): bs}
    for k in range(K):
        nu = {(k + 1, 0): beta * np.ones(D)}
        nv = {(k + 1, 0): np.zeros(D)}
        for j in range(k + 1):
            nu[(k + 1, j + 1)] = sb * u[(k, j)]
            nv[(k + 1, j + 1)] = sb * v[(k, j)]
        u.update(nu)
        v.update(nv)

    c = np.zeros(K + 1)
    d = np.zeros(K + 1)
    for k in range(K + 1):
        for j in range(k + 1):
            c[j] += float(u[(k, j)] @ w[k])
            d[j] += float(v[(k, j)] @ w[k])
    e = d[0] + float(b_mlp[0])
    return c, d, e


def _shift_matrix():
    sh = np.zeros((128, 128), dtype=np.float32)
    idx = np.arange(64)
    sh[idx, idx + 64] = 1.0
    sh[idx + 64, idx] = 1.0
    return sh


def _qview(ap):
    """[P, S] -> [P, q, f] with f in 0..3 (col = 4q+f)."""
    return ap.rearrange("p (q f) -> p q f", f=4)


def build_nc():
    nc = bacc.Bacc("TRN2", target_bir_lowering=False, debug=False)

    # adj pre-packed on host into the quad layout, fp8 e3m4 scaled x16:
    # adjq[b, p, q*128 + cb*64 + m] = 16*adj[b, 4q + sigma(pb,cb), n, m],
    # p = 64*pb + n, sigma = [[1,0],[2,3]][pb][cb]
    adj_l = nc.dram_tensor("adj", [BL, 128, Q * 128], ADJ_DT,
                           kind="ExternalInput")
    # x (fp16); pre planes j=0,1,2 are computed on-device as
    # pre_j = coefs_c[j] * x + coefs_d[j] (j=3 ships pre-staged as v3init)
    x_l = nc.dram_tensor("x16", [BL, N, S], V_DT, kind="ExternalInput")
    coefs_l = nc.dram_tensor("coefs", [128, 8], f32, kind="ExternalInput")
    v3_l = nc.dram_tensor("v3init", [BL, 128, S], V_DT, kind="ExternalInput")
    shift16 = nc.dram_tensor("shift16", [128, 128], V_DT, kind="ExternalInput")
    ident16 = nc.dram_tensor("ident16", [128, 128], V_DT, kind="ExternalInput")
    out_l = nc.dram_tensor("out", [BL, N, S], V_DT, kind="ExternalOutput")

    with tile.TileContext(nc) as tc:
        with (
            tc.tile_pool(name="singles", bufs=1) as singles,
            tc.tile_pool(name="o_pool", bufs=2) as o_pool,
            tc.tile_pool(name="psb_pool", bufs=1, space=bass.MemorySpace.PSUM)
            as psb_pool,
        ):
            sh16 = singles.tile([128, 128], V_DT, tag="sh16", name="sh16")
            id16 = singles.tile([128, 128], V_DT, tag="id16", name="id16")

            # chain-vector tiles, double-buffered by batch parity so
            # consecutive batches pipeline; complementary halves stay zero
            # (memset once; transitions only ever write the class halves).
            V = {}
            T16 = {}
            TF = {}
            for par in (0, 1):
                for k in (1, 2, 3):
                    V[par, k] = singles.tile([128, S], V_DT,
                                             tag=f"v{par}{k}", name=f"v{par}{k}")
                T16[par] = singles.tile([128, S], V_DT,
                                        tag=f"t16_{par}", name=f"t16_{par}")
                TF[par] = singles.tile([128, S], V_DT,
                                       tag=f"tf_{par}", name=f"tf_{par}")

            # ---- all input DMAs up front ----
            # adj: batch 0 in 4 chunks (earliest compute start), rest halved;
            # all on the SP queue.
            adj_t = [singles.tile([128, Q * 128], ADJ_DT,
                                  tag=f"adj{b}", name=f"adj{b}")
                     for b in range(BL)]
            # adj all on the SP queue: its FIFO gives a strict global
            # arrival order matched to the pipeline's consumption order
            # (batch 0 first, chunked to slice granularity for the earliest
            # start; sh16/id16 after the first two chunks).
            off = 0
            for i, nq in enumerate((14, 14, 28, 28)):
                nc.sync.dma_start(out=adj_t[0][:, off * 128:(off + nq) * 128],
                                  in_=adj_l[0][:, off * 128:(off + nq) * 128])
                off += nq
                if i == 1:
                    nc.sync.dma_start(sh16[:], shift16[:])
                    nc.sync.dma_start(id16[:], ident16[:])
            half = Q * 128 // 2
            for b in (1, 2, 3):
                nc.sync.dma_start(out=adj_t[b][:, :half], in_=adj_l[b][:, :half])
                nc.sync.dma_start(out=adj_t[b][:, half:], in_=adj_l[b][:, half:])

            # v3 starts + x planes (mirrored) on the gpsimd (Pool) queue,
            # batch-0 parity dependencies first, memsets after
            x_t = [singles.tile([128, S], V_DT, tag=f"x{b}", name=f"x{b}")
                   for b in range(BL)]
            for par in (0, 1):
                nc.gpsimd.dma_start(out=V[par, 3][:, :], in_=v3_l[par][:, :])
            def x_dma(b):
                srcap = bass.AP(x_l, b * N * S, [[S, N], [1, S]])
                nc.gpsimd.dma_start(out=x_t[b][0:64, :], in_=srcap)
                nc.gpsimd.dma_start(out=x_t[b][64:128, :], in_=srcap)

            x_dma(0)
            x_dma(1)
            for par in (0, 1):
                nc.gpsimd.memset(V[par, 1][:], 0.0)
                nc.gpsimd.memset(V[par, 2][:], 0.0)
                nc.gpsimd.memset(T16[par][:], 0.0)
                nc.gpsimd.memset(TF[par][:], 0.0)


            # pre planes computed on-device: pre_j = c_j' * x + d_j'
            # (coefficients are input data, shipped as per-partition APs)
            coefs = singles.tile([128, 8], f32, tag="coefs", name="coefs")
            nc.scalar.dma_start(out=coefs[:], in_=coefs_l[:])
            pre_t = [singles.tile([128, MP_LAYERS * S], V_DT,
                                  tag=f"pre{b}", name=f"pre{b}")
                     for b in range(BL)]

            def pre_compute(b, planes=range(MP_LAYERS)):
                for j in planes:
                    nc.scalar.activation(
                        pre_t[b][:, j * S:(j + 1) * S], x_t[b][:, :],
                        mybir.ActivationFunctionType.Identity,
                        bias=coefs[:, 4 + j:5 + j],
                        scale=coefs[:, j:j + 1])

            pre_compute(0)
            pre_compute(1)

            def pre_view(t, lo, hi, j, qs, fsl):
                v = t[lo:hi, :].rearrange("p (j q f) -> p j q f",
                                          j=MP_LAYERS, f=4)
                return v[:, j, qs, fsl]

            # ---- 3-deep skewed software pipeline ---------------------------
            # Cross-engine handoffs (psum -> DVE -> PE -> Scalar) measure
            # ~0.7us each, so a transition chain can never finish inside its
            # own pass window. Step s runs pass 3 of batch s, pass 2 of
            # batch s-1, pass 1 of batch s-2 and the deferred final fixup of
            # batch s-3: each transition has most of a step of slack before
            # its result is consumed.
            #
            # Transition layout: the staging tile T16 holds the crossed
            # class pair (cols 4q+0 / 4q+2) in block A (cols [0:168), pair-
            # interleaved 2q+t) and the direct pair (4q+1 / 4q+3) in block B
            # (cols [168:336)). One DVE add per block stages psum+pre for
            # BOTH partition halves (the useless half of each psum column is
            # staged too and filtered by the copies). Then one PE matmul per
            # block - the 64<->64 partition swap for block A, identity for
            # block B - lands everything in one psum region at canonical
            # halves, and two affine Scalar copies write V (top: cols
            # 4q+{0,1}, bottom: 4q+{2,3}; complementary V halves stay zero
            # from the one-time memset).
            psums = {}
            shps = {}
            Os = {}

            # PSUM is 8 banks x 2KB/partition, allocated at bank granularity:
            # banks 0-3 hold 4 rotating pass-psum slots ([128,336] f32);
            # banks 4-7 hold 4 rotating transition slots ([128,336] f32).
            psb = [psb_pool.tile([128, 512], f32, tag=f"psb{i}",
                                 name=f"psb{i}") for i in range(8)]
            _ctr = {"ps": 0, "sh": 0}

            def alloc_ps():
                s = _ctr["ps"] % 4
                _ctr["ps"] += 1
                return psb[s][:, 0:336]

            def alloc_sh():
                s = _ctr["sh"] % 4
                _ctr["sh"] += 1
                return psb[4 + s][:, 0:336]

            def p_slice(b, k, q0, q1):
                key = (b, k)
                if key not in psums:
                    psums[key] = alloc_ps()
                ps = psums[key]
                for q in range(q0, q1):
                    nc.tensor.matmul(
                        ps[:, 4 * q:4 * q + 4],
                        adj_t[b][:, 128 * q:128 * (q + 1)],
                        V[b % 2, k][:, 4 * q:4 * q + 4],
                        start=True, stop=True,
                    )

            def blk(t, base, q0, q1):
                # pair-interleaved block view [128, q, t] (col base+2q+t)
                return t[:, base + 2 * q0:base + 2 * q1].rearrange(
                    "p (q c) -> p q c", c=2)

            def stage(b, k, q0, q1):
                # DVE: stage psum_k + pre_{k-1}: crossed pair -> block A,
                # direct pair -> block B
                p = _qview(psums[b, k][:, :])
                qs = slice(q0, q1)
                t = T16[b % 2]
                nc.vector.tensor_add(
                    blk(t, 0, q0, q1)[:, :, :], p[:, qs, 0:3:2],
                    pre_view(pre_t[b], 0, 128, k - 1, qs, slice(0, 3, 2)))
                nc.vector.tensor_add(
                    blk(t, 168, q0, q1)[:, :, :], p[:, qs, 1:4:2],
                    pre_view(pre_t[b], 0, 128, k - 1, qs, slice(1, 4, 2)))

            def fixup(b, k, q0, q1):
                # PE: swap block A, pass block B through (identity)
                key = (b, k)
                if key not in shps:
                    shps[key] = alloc_sh()
                sv = shps[key]
                t = T16[b % 2]
                nc.tensor.matmul(sv[:, 2 * q0:2 * q1], sh16[:],
                                 t[:, 2 * q0:2 * q1],
                                 start=True, stop=True,
                                 skip_group_check=True)
                nc.tensor.matmul(sv[:, 168 + 2 * q0:168 + 2 * q1], id16[:],
                                 t[:, 168 + 2 * q0:168 + 2 * q1],
                                 start=True, stop=True,
                                 skip_group_check=True)

            def copies(b, k, q0, q1):
                # Scalar: svv[p, q, c, t]: c = block (A/B), t = within-pair
                vn = _qview(V[b % 2, k - 1][:, :])
                qs = slice(q0, q1)
                svv = shps[b, k][:, :].rearrange("p (c q t) -> p q c t",
                                                 c=2, t=2)
                nc.scalar.copy(vn[0:64, qs, 0:2], svv[0:64, qs, :, 0])
                nc.scalar.copy(vn[64:128, qs, 2:4], svv[64:128, qs, :, 1])

            def f_stage(b, q0, q1):
                # DVE: final staging: block A = (f0, f3) for the swap,
                # block B = (f1, f2) pass-through
                p1 = _qview(psums[b, 1][:, :])
                qs = slice(q0, q1)
                t = TF[b % 2]
                nc.vector.tensor_add(
                    blk(t, 0, q0, q1)[:, :, :], p1[:, qs, 0:4:3],
                    pre_view(pre_t[b], 0, 128, 0, qs, slice(0, 4, 3)))
                nc.vector.tensor_add(
                    blk(t, 168, q0, q1)[:, :, :], p1[:, qs, 1:3],
                    pre_view(pre_t[b], 0, 128, 0, qs, slice(1, 3)))

            def f_fixup(b, q0, q1):
                key = (b, 'F')
                if key not in shps:
                    shps[key] = alloc_sh()
                sv = shps[key]
                t = TF[b % 2]
                nc.tensor.matmul(sv[:, 2 * q0:2 * q1], sh16[:],
                                 t[:, 2 * q0:2 * q1],
                                 start=True, stop=True,
                                 skip_group_check=True)
                nc.tensor.matmul(sv[:, 168 + 2 * q0:168 + 2 * q1], id16[:],
                                 t[:, 168 + 2 * q0:168 + 2 * q1],
                                 start=True, stop=True,
                                 skip_group_check=True)

            def f_copies(b, q0, q1):
                # after the swap all four classes are useful at the top:
                # f0 = (A, t0), f1 = (B, t0), f2 = (B, t1), f3 = (A, t1)
                ov = _qview(Os[b][:, :])
                qs = slice(q0, q1)
                svv = shps[b, 'F'][:, :].rearrange("p (c q t) -> p q c t",
                                                   c=2, t=2)
                nc.scalar.copy(ov[:, qs, 0:2], svv[0:64, qs, :, 0])
                nc.scalar.copy(ov[:, qs, 2:3], svv[0:64, qs, 1, 1])
                nc.scalar.copy(ov[:, qs, 3:4], svv[0:64, qs, 0, 1])

            SL = Q // 6                  # pass-3 slice (14 quads)
            H = Q // 2                   # pass-2/1 and trans-3 half split

            for s in range(BL + 2):
                b3 = s if s < BL else None            # pass-3 batch
                b2 = s - 1 if 0 <= s - 1 < BL else None   # pass-2 batch
                b1 = s - 2 if 0 <= s - 2 < BL else None   # pass-1 batch
                bf = s - 3 if 0 <= s - 3 < BL else None   # final-fixup batch
                if b1 is not None:
                    Os[b1] = o_pool.tile([64, S], V_DT, tag="o",
                                         name=f"o{b1}")

                if b3 is not None:
                    # front-load pass-3 slices: during pipeline fill the
                    # pass-2 head blocks the in-order PE queue on batch
                    # b2's first transition chain, so give it filler first
                    p_slice(b3, 3, 0, SL)
                    p_slice(b3, 3, SL, 2 * SL)
                    p_slice(b3, 3, 2 * SL, 3 * SL)
                    stage(b3, 3, 0, H)                # DVE (ready first now)
                if b2 is not None:
                    p_slice(b2, 2, 0, H)
                if b3 is not None:
                    p_slice(b3, 3, 3 * SL, 4 * SL)
                if b2 is not None:
                    p_slice(b2, 2, H, Q)
                    stage(b2, 2, 0, Q)                # DVE
                if bf is not None and bf < BL - 1:
                    f_fixup(bf, 0, Q)
                    f_copies(bf, 0, Q)                # Scalar
                    nc.sync.dma_start(out=out_l[bf], in_=Os[bf][:])
                if b3 is not None:
                    p_slice(b3, 3, 4 * SL, 5 * SL)
                    p_slice(b3, 3, 5 * SL, Q)
                    stage(b3, 3, H, Q)                # DVE
                if b1 is not None:
                    p_slice(b1, 1, 0, H)
                if b3 is not None:
                    fixup(b3, 3, 0, H)
                    copies(b3, 3, 0, H)               # Scalar
                if b1 is not None:
                    p_slice(b1, 1, H, Q)
                if b2 is not None:
                    fixup(b2, 2, 0, Q)
                if b3 is not None:
                    fixup(b3, 3, H, Q)
                    copies(b3, 3, H, Q)               # Scalar
                if b2 is not None:
                    copies(b2, 2, 0, Q)               # Scalar
                if b1 is not None and b1 < BL - 1:
                    f_stage(b1, 0, Q)                 # DVE
                if b1 is not None and b1 == BL - 1:
                    # last batch: run its final chain split in halves right
                    # behind its own pass-1 so the drain tail stays short
                    f_stage(b1, 0, H)
                    f_fixup(b1, 0, H)
                    f_copies(b1, 0, H)
                    nc.sync.dma_start(out=out_l[b1][:, 0:2 * H],
                                      in_=Os[b1][:, 0:2 * H])
                    f_stage(b1, H, Q)
                    f_fixup(b1, H, Q)
                    f_copies(b1, H, Q)
                    nc.sync.dma_start(out=out_l[b1][:, 2 * H:],
                                      in_=Os[b1][:, 2 * H:])
                if b3 is not None and b3 + 2 < BL:
                    # refresh V3 for batch b3+2 (same parity); the DMA waits
                    # for this step's pass-3 reads to drain
                    nc.gpsimd.dma_start(out=V[b3 % 2, 3][:, :],
                                        in_=v3_l[b3 + 2][:, :])
                # lazy pre planes for later batches, most-urgent (j=2) first,
                # at end-of-step scalar positions so copies are never delayed
                if s == 0:
                    x_dma(2)
                    pre_compute(2, (2,))
                elif s == 1:
                    x_dma(3)
                    pre_compute(2, (1, 0))
                    pre_compute(3, (2,))
                elif s == 2:
                    pre_compute(3, (1, 0))


    nc.finalize()
    return nc


_NC_CACHE = None


def _get_nc():
    global _NC_CACHE
    if _NC_CACHE is None:
        _NC_CACHE = build_nc()
    return _NC_CACHE


def _pack_adj(adj):
    """[B, S, N, N] f32 -> [B, 128, Q*128] fp8 (x16) quad layout."""
    sigma = np.array([[1, 0], [2, 3]])  # [pb][cb]
    # s_idx[q, pb, cb] = 4q + sigma[pb, cb]
    s_idx = 4 * np.arange(Q)[:, None, None] + sigma[None, :, :]
    a = adj[:, s_idx]                      # [B, Q, 2pb, 2cb, n, m]
    a = a.transpose(0, 2, 4, 1, 3, 5)      # [B, pb, n, Q, cb, m]
    return np.ascontiguousarray(
        (a.reshape(B, 128, Q * 128) * ADJ_SCALE).astype(ADJ_NP))


def _prepare_in_maps(x, adj, w_start, b_start, w_mlp, b_mlp):
    c, d, e = _coefficients(np.asarray(w_start), np.asarray(b_start),
                            np.asarray(w_mlp), np.asarray(b_mlp))
    x = np.asarray(x, dtype=np.float32)
    adj = _pack_adj(np.asarray(adj, dtype=np.float32))
    # pre_j is computed on-device as c_j' * x + d_j'; 16^(3-j) compensates
    # the x16 fp8 scale of adj accumulating through the remaining (3-j)
    # matmul passes (e already folds in d[0] + b_mlp).
    coefs = np.zeros((128, 8), dtype=np.float32)
    for j in range(MP_LAYERS):
        coefs[:, j] = c[j] * ADJ_SCALE ** (MP_LAYERS - j)
        coefs[:, 4 + j] = ((e if j == 0 else d[j])
                           * ADJ_SCALE ** (MP_LAYERS - j))
    x16 = x.astype(V_NP)
    # v3init[b]: pre3 staged at each chain's input half (j0,j1 top; j2,j3
    # bottom), zeros elsewhere
    pre3 = (c[MP_LAYERS] * x + d[MP_LAYERS]).astype(V_NP)  # [B, N, S]
    p3q = pre3.reshape(B, N, Q, 4)
    v3 = np.zeros((B, 128, Q, 4), dtype=V_NP)
    v3[:, 0:64, :, 0:2] = p3q[:, :, :, 0:2]
    v3[:, 64:128, :, 2:4] = p3q[:, :, :, 2:4]
    v3 = v3.reshape(B, 128, S)
    sh = _shift_matrix().astype(V_NP)
    ident = np.eye(128, dtype=np.float32).astype(V_NP)
    in_maps = []
    for i in range(NCORES):
        sl = slice(i * BL, (i + 1) * BL)
        in_maps.append({
            "adj": np.ascontiguousarray(adj[sl]),
            "x16": np.ascontiguousarray(x16[sl]),
            "coefs": coefs,
            "v3init": np.ascontiguousarray(v3[sl]),
            "shift16": sh,
            "ident16": ident,
        })
    return in_maps


def run_spmd(inputs, trace=False, **kw):
    in_maps = _prepare_in_maps(**inputs)
    res = run_bass_kernel_spmd(_get_nc(), in_maps,
                               core_ids=list(range(NCORES)), trace=trace, **kw)
    out = np.concatenate([r["out"] for r in res.results], axis=0)
    return out.astype(np.float32) / OUT_DESCALE, res


def kernel(**inputs):
    out, _ = run_spmd(inputs)
    return out.astype(np.float32)


if __name__ == "__main__":
    # quick smoke test against a numpy oracle
    rng = np.random.default_rng(0)
    x = rng.standard_normal((B, N, S), dtype=np.float32)
    adj = rng.random((B, S, N, N), dtype=np.float32)
    adj /= adj.sum(-1, keepdims=True)
    w_start = rng.standard_normal((1, D)).astype(np.float32)
    b_start = (rng.standard_normal(D) * 0.01).astype(np.float32)
    w_mlp = (rng.standard_normal(((MP_LAYERS + 1) * D, 1)) /
             np.sqrt((MP_LAYERS + 1) * D)).astype(np.float32)
    b_mlp = (rng.standard_normal(1) * 0.01).astype(np.float32)

    got = kernel(x=x, adj=adj, w_start=w_start, b_start=b_start,
                 w_mlp=w_mlp, b_mlp=b_mlp)

    h = x[..., None] * w_start[0] + b_start
    outs = [h]
    a = np.transpose(adj, (0, 2, 3, 1))
    for _ in range(MP_LAYERS):
        conv = np.einsum('bnsc,bnms->bmsc', h, a, optimize=True)
        h = PROPBETA * x[..., None] + (1 - PROPBETA) * conv
        outs.append(h)
    hc = np.concatenate(outs, axis=-1)
    want = (hc @ w_mlp)[..., 0] + b_mlp[0]

    aerr = np.abs(got - want)
    print("max abs err:", aerr.max(),
          "normalized:", aerr.max() / np.abs(want).max())
